# revision 2
# baseline (speedup 1.0000x reference)
"""AQT-style int8 fake-quant 3x3 conv (SAME), NHWC 32x56x56x256 -> 32x56x56x256.

Strategy (8 NeuronCores, data-parallel over batch):
  - Host: per-example quantize lhs, per-out-channel quantize rhs (exact
    integers in [-127,127] => exactly representable in bf16), pad to a
    58x58 halo and transpose to channel-major [cic,128,58*58] bf16.
  - Device (per core, 4 images): conv as 9-tap shifted matmuls on the
    TensorEngine, K = 3*3*256 contracted in 18 chunks of 128 into PSUM
    (f32, exact), dequant on VectorE with fused per-(image,channel)
    scale, DMA out channel-major f32.
  - Host: gather + transpose back to NHWC.

Raw Bass (explicit semaphores); the Tile framework's tail drain emits
multi-wait CTRL instructions this walrus build rejects.
"""

import sys

import numpy as np
import ml_dtypes

if "/opt/trn_rl_repo" not in sys.path:
    sys.path.insert(0, "/opt/trn_rl_repo")

import concourse.bass as bass
import concourse.mybir as mybir
from concourse.bass_utils import run_bass_kernel_spmd

_QMAX = 127.0

N, H, W, C = 32, 56, 56, 256
KH = KW = 3
NCORES = 8
NPER = N // NCORES          # 4 images per core
PH, PW = H + 2, W + 2       # 58x58 padded
NPAD = PH * PW              # 3364
NPIX = H * W                # 3136
RPT = 8                     # output rows per PSUM tile
NRT = H // RPT              # 7 row tiles per image
FREE = RPT * W              # 448 output pixels per matmul
NCIC = C // 128             # 2 input-channel chunks
NCOC = C // 128             # 2 output-channel chunks
NTAP = KH * KW              # 9
KSTEPS = NTAP * NCIC        # 18 matmuls per PSUM tile
TILES_PER_CORE = NPER * NCOC * NRT  # 56
NPSUM = 8                   # PSUM banks cycled

_BF16 = mybir.dt.bfloat16
_F32 = mybir.dt.float32


def _build_nc():
    nc = bass.Bass("TRN2", num_devices=NCORES)

    qlhs_ext = nc.declare_dram_parameter(
        "qlhs", [NPER, NCIC, 128, NPAD], _BF16, isOutput=False)
    qw_ext = nc.declare_dram_parameter(
        "qw", [NCIC, 128, NTAP * NCOC * 128], _BF16, isOutput=False)
    sc_ext = nc.declare_dram_parameter(
        "sc", [128, NCOC * NPER], _F32, isOutput=False)
    out_ext = nc.declare_dram_parameter(
        "out", [NPER, NCOC, 128, NPIX], _F32, isOutput=True)

    from contextlib import ExitStack
    with ExitStack() as ctx:
        # SBUF residency (per partition): qlhs 8*3364*2B = 53.8KB,
        # w 2*2304*2B = 9.2KB, out staging 56*448*4B = 100KB, sc 32B.
        w_sb = ctx.enter_context(
            nc.sbuf_tensor("w_sb", [128, NCIC * NTAP * NCOC * 128], _BF16))
        x_sb = [
            [ctx.enter_context(nc.sbuf_tensor(f"x_sb{i}_{c}", [128, NPAD], _BF16))
             for c in range(NCIC)]
            for i in range(NPER)
        ]
        o_sb = ctx.enter_context(
            nc.sbuf_tensor("o_sb", [128, TILES_PER_CORE * FREE], _F32))
        sc_sb = ctx.enter_context(nc.sbuf_tensor("sc_sb", [128, NCOC * NPER], _F32))
        ps = [ctx.enter_context(nc.psum_tensor(f"ps{i}", [128, FREE], _F32))
              for i in range(NPSUM)]

        wsem = ctx.enter_context(nc.semaphore("wsem"))
        scsem = ctx.enter_context(nc.semaphore("scsem"))
        qsem = [ctx.enter_context(nc.semaphore(f"qsem{i}")) for i in range(NPER)]
        mmsem = ctx.enter_context(nc.semaphore("mmsem"))
        dqsem = ctx.enter_context(nc.semaphore("dqsem"))
        osem = ctx.enter_context(nc.semaphore("osem"))

        block = ctx.enter_context(nc.Block())

        # tile index t decodes as (img, coc, rt), rt fastest
        def decode(t):
            img, r = divmod(t, NCOC * NRT)
            coc, rt = divmod(r, NRT)
            return img, coc, rt

        @block.sync
        def _(sync):
            # weights + scales + all images in, large contiguous DMAs
            for cic in range(NCIC):
                sync.dma_start(
                    w_sb[:, cic * NTAP * NCOC * 128:(cic + 1) * NTAP * NCOC * 128],
                    qw_ext[cic],
                ).then_inc(wsem, 16)
            sync.dma_start(sc_sb[:], sc_ext[:]).then_inc(scsem, 16)
            for img in range(NPER):
                for cic in range(NCIC):
                    sync.dma_start(
                        x_sb[img][cic][:], qlhs_ext[img, cic]
                    ).then_inc(qsem[img], 16)
            # stream results out as soon as each tile is dequantized
            for t in range(TILES_PER_CORE):
                img, coc, rt = decode(t)
                sync.wait_ge(dqsem, t + 1)
                sync.dma_start(
                    out_ext[img, coc][:, rt * FREE:(rt + 1) * FREE],
                    o_sb[:, t * FREE:(t + 1) * FREE],
                ).then_inc(osem, 16)
            sync.wait_ge(osem, TILES_PER_CORE * 16)

        @block.tensor
        def _(tensor):
            tensor.wait_ge(wsem, NCIC * 16)
            for t in range(TILES_PER_CORE):
                img, coc, rt = decode(t)
                if t % (NCOC * NRT) == 0:
                    tensor.wait_ge(qsem[img], NCIC * 16)
                if t >= NPSUM:
                    # PSUM bank reuse: wait for dequant of tile t-NPSUM
                    tensor.wait_ge(dqsem, t - NPSUM + 1)
                k = 0
                mm = None
                for dy in range(KH):
                    for dx in range(KW):
                        for cic in range(NCIC):
                            w_ap = w_sb[:, (cic * NTAP * NCOC
                                            + (dy * KW + dx) * NCOC + coc)
                                        * 128:][:, :128]
                            x_ap = (x_sb[img][cic][:]
                                    .rearrange("p (r c) -> p r c", c=PW)
                                    [:, rt * RPT + dy: rt * RPT + dy + RPT,
                                     dx: dx + W])
                            mm = nc.tensor.matmul(
                                ps[t % NPSUM][:], w_ap, x_ap,
                                start=(k == 0), stop=(k == KSTEPS - 1))
                            k += 1
                mm.then_inc(mmsem, 1)

        @block.vector
        def _(vector):
            vector.wait_ge(scsem, 16)
            for t in range(TILES_PER_CORE):
                img, coc, rt = decode(t)
                vector.wait_ge(mmsem, t + 1)
                nc.vector.tensor_scalar_mul(
                    o_sb[:, t * FREE:(t + 1) * FREE],
                    ps[t % NPSUM][:],
                    sc_sb[:, coc * NPER + img: coc * NPER + img + 1],
                ).then_inc(dqsem, 1)

    return nc


_NC_CACHE = None


def kernel(lhs: np.ndarray, rhs: np.ndarray) -> np.ndarray:
    global _NC_CACHE
    lhs = np.asarray(lhs, dtype=np.float32)
    rhs = np.asarray(rhs, dtype=np.float32)
    assert lhs.shape == (N, H, W, C) and rhs.shape == (KH, KW, C, C)

    # --- host-side quantization (exact integers; replicated scales) ---
    amax_l = np.abs(lhs).max(axis=(1, 2, 3))                  # [N]
    s_l = np.maximum(amax_l, 1e-6) / _QMAX
    ql = np.rint(lhs / s_l[:, None, None, None]).astype(np.float32)

    amax_r = np.abs(rhs).max(axis=(0, 1, 2))                  # [C]
    s_r = np.maximum(amax_r, 1e-6) / _QMAX
    qr = np.rint(rhs / s_r[None, None, None, :]).astype(np.float32)

    # lhs -> per-core [NPER, NCIC, 128, 58*58] bf16, zero halo
    qpad = np.zeros((N, PH, PW, C), dtype=np.float32)
    qpad[:, 1:H + 1, 1:W + 1, :] = ql
    # [N, PH, PW, C] -> [N, C, PH*PW] -> [N, NCIC, 128, NPAD]
    qlhs_dev = (qpad.transpose(0, 3, 1, 2)
                .reshape(N, NCIC, 128, NPAD)
                .astype(ml_dtypes.bfloat16))

    # rhs -> [NCIC, 128, NTAP*NCOC*128] bf16 (free idx = (tap*NCOC+coc)*128+co)
    qw_dev = (qr.reshape(NTAP, NCIC, 128, C)
              .transpose(1, 2, 0, 3)
              .reshape(NCIC, 128, NTAP * C)
              .astype(ml_dtypes.bfloat16))

    # fused dequant scale per (image, out-channel): sc[co128, coc*NPER+img]
    s_r2 = s_r.reshape(NCOC, 128)

    nc = _NC_CACHE
    if nc is None:
        nc = _NC_CACHE = _build_nc()

    in_maps = []
    for core in range(NCORES):
        s_l_core = s_l[core * NPER:(core + 1) * NPER]         # [NPER]
        sc = np.empty((128, NCOC * NPER), dtype=np.float32)
        for coc in range(NCOC):
            sc[:, coc * NPER:(coc + 1) * NPER] = (
                s_r2[coc][:, None] * s_l_core[None, :])
        in_maps.append({
            "qlhs": qlhs_dev[core * NPER:(core + 1) * NPER],
            "qw": qw_dev,
            "sc": sc,
        })

    res = run_bass_kernel_spmd(nc, in_maps, list(range(NCORES)))

    # gather: [NPER, NCOC, 128, NPIX] f32 -> NHWC
    outs = []
    for core in range(NCORES):
        o = res.results[core]["out"]                          # [4, 2, 128, 3136]
        outs.append(o.reshape(NPER, C, NPIX).transpose(0, 2, 1)
                    .reshape(NPER, H, W, C))
    return np.concatenate(outs, axis=0).astype(np.float32)


# revision 4
# speedup vs baseline: 1.0187x; 1.0187x over previous
"""AQT-style int8 fake-quant 3x3 conv (SAME), NHWC 32x56x56x256 -> 32x56x56x256.

Strategy (8 NeuronCores, data-parallel over batch):
  - Host: per-example quantize lhs, per-out-channel quantize rhs (exact
    integers in [-127,127] => exactly representable in bf16), pad to a
    58x58 halo and transpose to channel-major [cic,128,58*58] bf16.
  - Device (per core, 4 images): conv as 9-tap shifted matmuls on the
    TensorEngine, K = 3*3*256 contracted in 18 chunks of 128 into PSUM
    (f32, exact), dequant on VectorE with fused per-(image,channel)
    scale, DMA out channel-major f32.
  - Host: gather + transpose back to NHWC.

Raw Bass (explicit semaphores); the Tile framework's tail drain emits
multi-wait CTRL instructions this walrus build rejects.

Perf notes (HWDGE queue is FIFO per engine, ~390 GB/s aggregate):
  - DMA issue order = completion order, so the startup-critical bytes
    (weights for coc0 + the first 10 padded rows of image 0) go first.
  - ~80 tiny matmuls on garbage data prewarm the PE HAM clock gate
    (1.2 -> 2.4 GHz) while the first DMAs land.
  - The last tile's dequant+store is split in half to shorten the tail.
"""

import sys

import numpy as np
import ml_dtypes

if "/opt/trn_rl_repo" not in sys.path:
    sys.path.insert(0, "/opt/trn_rl_repo")

import concourse.bass as bass
import concourse.mybir as mybir
from concourse.bass_utils import run_bass_kernel_spmd

_QMAX = 127.0

N, H, W, C = 32, 56, 56, 256
KH = KW = 3
NCORES = 8
NPER = N // NCORES          # 4 images per core
PH, PW = H + 2, W + 2       # 58x58 padded
NPAD = PH * PW              # 3364
NPIX = H * W                # 3136
RPT = 8                     # output rows per PSUM tile
NRT = H // RPT              # 7 row tiles per image
FREE = RPT * W              # 448 output pixels per matmul
NCIC = C // 128             # 2 input-channel chunks
NCOC = C // 128             # 2 output-channel chunks
NTAP = KH * KW              # 9
KSTEPS = NTAP * NCIC        # 18 matmuls per PSUM tile
TILES_PER_CORE = NPER * NCOC * NRT  # 56
NPSUM = 8                   # PSUM banks cycled
NWARM = 80                  # prewarm matmuls (N=64) to flip HAM to 2.4 GHz

# img0 row-chunk boundaries (padded rows): tile rt needs rows <= rt*8+9
ROWC = [0, 10 * PW, 34 * PW, NPAD]   # chunks A=[rows 0-9] B=[10-33] C=[34-57]

_BF16 = mybir.dt.bfloat16
_F32 = mybir.dt.float32


def _build_nc():
    nc = bass.Bass("TRN2", num_devices=NCORES)

    qlhs_ext = nc.declare_dram_parameter(
        "qlhs", [NPER, NCIC, 128, NPAD], _BF16, isOutput=False)
    # qw free layout: ((cic*NCOC + coc)*NTAP + tap)*128 + co
    qw_ext = nc.declare_dram_parameter(
        "qw", [NCIC, NCOC, 128, NTAP * 128], _BF16, isOutput=False)
    sc_ext = nc.declare_dram_parameter(
        "sc", [128, NCOC * NPER], _F32, isOutput=False)
    out_ext = nc.declare_dram_parameter(
        "out", [NPER, NCOC, 128, NPIX], _F32, isOutput=True)

    from contextlib import ExitStack
    with ExitStack() as ctx:
        # SBUF residency (per partition): qlhs 8*3364*2B = 53.8KB,
        # w 2*2304*2B = 9.2KB, out staging 56*448*4B = 100KB, sc 32B.
        w_sb = ctx.enter_context(
            nc.sbuf_tensor("w_sb", [128, NCIC * NCOC * NTAP * 128], _BF16))
        x_sb = [
            [ctx.enter_context(nc.sbuf_tensor(f"x_sb{i}_{c}", [128, NPAD], _BF16))
             for c in range(NCIC)]
            for i in range(NPER)
        ]
        o_sb = ctx.enter_context(
            nc.sbuf_tensor("o_sb", [128, TILES_PER_CORE * FREE], _F32))
        sc_sb = ctx.enter_context(nc.sbuf_tensor("sc_sb", [128, NCOC * NPER], _F32))
        ps = [ctx.enter_context(nc.psum_tensor(f"ps{i}", [128, FREE], _F32))
              for i in range(NPSUM)]

        wsem = [ctx.enter_context(nc.semaphore(f"wsem{i}")) for i in range(NCOC)]
        scsem = ctx.enter_context(nc.semaphore("scsem"))
        qsem = [ctx.enter_context(nc.semaphore(f"qsem{i}")) for i in range(NPER)]
        q0sem = [ctx.enter_context(nc.semaphore(f"q0sem{i}")) for i in range(3)]
        mmsem = ctx.enter_context(nc.semaphore("mmsem"))
        dqsem = ctx.enter_context(nc.semaphore("dqsem"))
        lastsem = ctx.enter_context(nc.semaphore("lastsem"))
        osem = ctx.enter_context(nc.semaphore("osem"))

        block = ctx.enter_context(nc.Block())

        # tile index t decodes as (img, coc, rt), rt fastest
        def decode(t):
            img, r = divmod(t, NCOC * NRT)
            coc, rt = divmod(r, NRT)
            return img, coc, rt

        def wslice(cic, coc, tap):
            col = ((cic * NCOC + coc) * NTAP + tap) * 128
            return w_sb[:, col:col + 128]

        LAST = TILES_PER_CORE - 1
        HFREE = FREE // 2

        @block.sync
        def _(sync):
            # ---- inputs, critical-first (per-engine queue is FIFO) ----
            # 1) img0 rows 0-9 + coc0 weights: unblocks tile 0 (~0.9MB)
            for cic in range(NCIC):
                sync.dma_start(
                    x_sb[0][cic][:, ROWC[0]:ROWC[1]],
                    qlhs_ext[0, cic][:, ROWC[0]:ROWC[1]],
                ).then_inc(q0sem[0], 16)
            for cic in range(NCIC):
                sync.dma_start(
                    w_sb[:, (cic * NCOC) * NTAP * 128:][:, :NTAP * 128],
                    qw_ext[cic, 0],
                ).then_inc(wsem[0], 16)
            # 2) rest of img0
            for chunk in (1, 2):
                for cic in range(NCIC):
                    sync.dma_start(
                        x_sb[0][cic][:, ROWC[chunk]:ROWC[chunk + 1]],
                        qlhs_ext[0, cic][:, ROWC[chunk]:ROWC[chunk + 1]],
                    ).then_inc(q0sem[chunk], 16)
            # 3) scales, coc1 weights, remaining images
            sync.dma_start(sc_sb[:], sc_ext[:]).then_inc(scsem, 16)
            for cic in range(NCIC):
                sync.dma_start(
                    w_sb[:, (cic * NCOC + 1) * NTAP * 128:][:, :NTAP * 128],
                    qw_ext[cic, 1],
                ).then_inc(wsem[1], 16)
            for img in range(1, NPER):
                for cic in range(NCIC):
                    sync.dma_start(
                        x_sb[img][cic][:], qlhs_ext[img, cic]
                    ).then_inc(qsem[img], 16)
            # ---- stream results out as each tile is dequantized ----
            for t in range(TILES_PER_CORE - 1):
                img, coc, rt = decode(t)
                sync.wait_ge(dqsem, t + 1)
                sync.dma_start(
                    out_ext[img, coc][:, rt * FREE:(rt + 1) * FREE],
                    o_sb[:, t * FREE:(t + 1) * FREE],
                ).then_inc(osem, 16)
            # last tile in two halves (shorter dequant->store tail)
            img, coc, rt = decode(LAST)
            for h in range(2):
                sync.wait_ge(lastsem, h + 1)
                sync.dma_start(
                    out_ext[img, coc][:, rt * FREE + h * HFREE:
                                      rt * FREE + (h + 1) * HFREE],
                    o_sb[:, LAST * FREE + h * HFREE:
                         LAST * FREE + (h + 1) * HFREE],
                ).then_inc(osem, 16)
            sync.wait_ge(osem, (TILES_PER_CORE + 1) * 16)

        @block.tensor
        def _(tensor):
            # HAM prewarm on garbage SBUF data; bank 7 is clobbered by
            # its first real accumulation group (start=True) later.
            for i in range(NWARM):
                nc.tensor.matmul(ps[NPSUM - 1][:, :64], w_sb[:, :128],
                                 x_sb[0][0][:, :64], start=True, stop=True)
            tensor.wait_ge(wsem[0], NCIC * 16)
            tensor.wait_ge(q0sem[0], NCIC * 16)
            for t in range(TILES_PER_CORE):
                img, coc, rt = decode(t)
                if t == NRT:                      # first coc=1 tile
                    tensor.wait_ge(wsem[1], NCIC * 16)
                if img == 0:
                    if t == 1:
                        tensor.wait_ge(q0sem[1], NCIC * 16)
                    elif t == 4:
                        tensor.wait_ge(q0sem[2], NCIC * 16)
                elif t % (NCOC * NRT) == 0:
                    tensor.wait_ge(qsem[img], NCIC * 16)
                if t >= NPSUM:
                    # PSUM bank reuse: wait for dequant of tile t-NPSUM
                    tensor.wait_ge(dqsem, t - NPSUM + 1)
                k = 0
                mm = None
                for dy in range(KH):
                    for dx in range(KW):
                        for cic in range(NCIC):
                            x_ap = (x_sb[img][cic][:]
                                    .rearrange("p (r c) -> p r c", c=PW)
                                    [:, rt * RPT + dy: rt * RPT + dy + RPT,
                                     dx: dx + W])
                            mm = nc.tensor.matmul(
                                ps[t % NPSUM][:], wslice(cic, coc, dy * KW + dx),
                                x_ap, start=(k == 0), stop=(k == KSTEPS - 1))
                            k += 1
                mm.then_inc(mmsem, 1)

        @block.vector
        def _(vector):
            vector.wait_ge(scsem, 16)
            for t in range(TILES_PER_CORE):
                img, coc, rt = decode(t)
                vector.wait_ge(mmsem, t + 1)
                scol = sc_sb[:, coc * NPER + img: coc * NPER + img + 1]
                if t < LAST:
                    nc.vector.tensor_scalar_mul(
                        o_sb[:, t * FREE:(t + 1) * FREE],
                        ps[t % NPSUM][:], scol,
                    ).then_inc(dqsem, 1)
                else:
                    for h in range(2):
                        nc.vector.tensor_scalar_mul(
                            o_sb[:, t * FREE + h * HFREE:
                                 t * FREE + (h + 1) * HFREE],
                            ps[t % NPSUM][:, h * HFREE:(h + 1) * HFREE], scol,
                        ).then_inc(lastsem, 1)

    return nc


_NC_CACHE = None


def kernel(lhs: np.ndarray, rhs: np.ndarray) -> np.ndarray:
    global _NC_CACHE
    lhs = np.asarray(lhs, dtype=np.float32)
    rhs = np.asarray(rhs, dtype=np.float32)
    assert lhs.shape == (N, H, W, C) and rhs.shape == (KH, KW, C, C)

    # --- host-side quantization (exact integers; replicated scales) ---
    amax_l = np.abs(lhs).max(axis=(1, 2, 3))                  # [N]
    s_l = np.maximum(amax_l, 1e-6) / _QMAX
    ql = np.rint(lhs / s_l[:, None, None, None]).astype(np.float32)

    amax_r = np.abs(rhs).max(axis=(0, 1, 2))                  # [C]
    s_r = np.maximum(amax_r, 1e-6) / _QMAX
    qr = np.rint(rhs / s_r[None, None, None, :]).astype(np.float32)

    # lhs -> per-core [NPER, NCIC, 128, 58*58] bf16, zero halo
    qpad = np.zeros((N, PH, PW, C), dtype=np.float32)
    qpad[:, 1:H + 1, 1:W + 1, :] = ql
    # [N, PH, PW, C] -> [N, C, PH*PW] -> [N, NCIC, 128, NPAD]
    qlhs_dev = (qpad.transpose(0, 3, 1, 2)
                .reshape(N, NCIC, 128, NPAD)
                .astype(ml_dtypes.bfloat16))

    # rhs -> [NCIC, NCOC, 128, NTAP*128] bf16 (free idx = tap*128+co)
    qw_dev = (qr.reshape(NTAP, NCIC, 128, NCOC, 128)
              .transpose(1, 3, 2, 0, 4)
              .reshape(NCIC, NCOC, 128, NTAP * 128)
              .astype(ml_dtypes.bfloat16))

    # fused dequant scale per (image, out-channel): sc[co128, coc*NPER+img]
    s_r2 = s_r.reshape(NCOC, 128)

    nc = _NC_CACHE
    if nc is None:
        nc = _NC_CACHE = _build_nc()

    in_maps = []
    for core in range(NCORES):
        s_l_core = s_l[core * NPER:(core + 1) * NPER]         # [NPER]
        sc = np.empty((128, NCOC * NPER), dtype=np.float32)
        for coc in range(NCOC):
            sc[:, coc * NPER:(coc + 1) * NPER] = (
                s_r2[coc][:, None] * s_l_core[None, :])
        in_maps.append({
            "qlhs": qlhs_dev[core * NPER:(core + 1) * NPER],
            "qw": qw_dev,
            "sc": sc,
        })

    res = run_bass_kernel_spmd(nc, in_maps, list(range(NCORES)))

    # gather: [NPER, NCOC, 128, NPIX] f32 -> NHWC
    outs = []
    for core in range(NCORES):
        o = res.results[core]["out"]                          # [4, 2, 128, 3136]
        outs.append(o.reshape(NPER, C, NPIX).transpose(0, 2, 1)
                    .reshape(NPER, H, W, C))
    return np.concatenate(outs, axis=0).astype(np.float32)
